# revision 11
# baseline (speedup 1.0000x reference)
"""Trainium2 Bass kernel for nn_AttentionalFlow (BiDAF-style attention flow).

Reference math (per batch b; c = embd_context [T=512, D=512],
q = embd_query [J=64, D=512], W = [3*D] split into wc, wq, wm):

  S[t,j] = c[t]·wc + q[j]·wq + sum_d c[t,d]*q[j,d]*wm[d]
         = sum_d c[t,d] * (q[j,d]*wm[d] + wc[d]) + q_term[j]
  P      = softmax_j(S)        (softmax is shift-invariant; |S| <~ 8 so we
                                can skip the max subtraction and exp directly)
  c2q    = P @ q
  e[t]   = exp(max_j S[t,j]);  q2c[d] = (sum_t e[t]*c[t,d]) / (sum_t e[t])
  G      = [c, c2q, c*c2q, c*q2c]   -> [T, 2048]

Sharding: data-parallel over batch. 32 batches / 8 cores = 4 batches per core.
W is tiny and replicated (pre-reshaped host-side to [128, 12]: col k holds
W[128k:128k+128]; cols 0-3 = wc, 4-7 = wq, 8-11 = wm chunks).
"""

import numpy as np

import concourse.bacc as bacc
import concourse.bass as bass
import concourse.tile as tile
from concourse import mybir
from concourse.bass_utils import run_bass_kernel_spmd
from concourse.masks import make_identity

F32 = mybir.dt.float32
F32R = mybir.dt.float32r

N_CORES = 8
B, T, J, D = 32, 512, 64, 512
BPC = B // N_CORES  # batches per core
NT = T // 128       # t-chunks of 128
NK = D // 128       # d-chunks of 128
GD = 4 * D          # output feature dim

# Use the fast fp32 matmul mode (float32r: full-rate at N>=512, vs 1/4-rate
# plain fp32) for the two large broadcast/attention matmuls whose operands we
# can materialize as rounded-fp32r tiles (walrus requires fp32r matmul inputs
# to be produced as fp32r). PSUM accumulation stays fp32.
USE_F32R = True
MMDT = F32R if USE_F32R else F32


def build_kernel():
    nc = bacc.Bacc()

    ctx_d = nc.dram_tensor("embd_context", [BPC, T, D], F32, kind="ExternalInput")
    qry_d = nc.dram_tensor("embd_query", [BPC, J, D], F32, kind="ExternalInput")
    wt_d = nc.dram_tensor("w_resh", [128, 12], F32, kind="ExternalInput")
    out_d = nc.dram_tensor("g_out", [BPC, T, GD], F32, kind="ExternalOutput")

    with tile.TileContext(nc) as tc:
        with (
            tc.tile_pool(name="singles", bufs=1) as singles,
            tc.tile_pool(name="gpool", bufs=2) as gpool,
            tc.tile_pool(name="spool", bufs=2) as spool,
            tc.tile_pool(name="small", bufs=4) as small,
            tc.tile_pool(name="ps_trans", bufs=2, space="PSUM") as ps_trans,
            tc.tile_pool(name="ps_s", bufs=2, space="PSUM") as ps_s,
            tc.tile_pool(name="ps_mm", bufs=2, space="PSUM") as ps_mm,
            tc.tile_pool(name="ps_vec", bufs=2, space="PSUM") as ps_vec,
        ):
            ident = singles.tile([128, 128], F32)
            make_identity(nc, ident)
            ones_row = singles.tile([1, 128], F32)
            nc.vector.memset(ones_row, 1.0)
            ones_col = singles.tile([128, 1], F32)
            nc.vector.memset(ones_col, 1.0)
            wt_sb = singles.tile([128, 12], F32)
            nc.sync.dma_start(out=wt_sb, in_=wt_d[:, :])

            for b in range(BPC):
                # --- load context into the G output tiles (slot 0 = c) ---
                g = [
                    gpool.tile([128, GD], F32, tag=f"g{i}", name=f"g{i}")
                    for i in range(NT)
                ]
                for i in range(NT):
                    nc.sync.dma_start(
                        out=g[i][:, 0:D],
                        in_=ctx_d[b, 128 * i : 128 * (i + 1), :],
                    )
                q_sb = spool.tile([J, D], F32, tag="q")
                nc.sync.dma_start(out=q_sb, in_=qry_d[b])
                if USE_F32R:
                    q_r = spool.tile([J, D], MMDT, tag="qr")
                    nc.vector.tensor_copy(q_r, q_sb)
                else:
                    q_r = q_sb

                # --- qT: [d, j] blocks via PE transpose ---
                qt_ps = ps_trans.tile([128, NK * J], F32, tag="trans")
                for k in range(NK):
                    nc.tensor.transpose(
                        qt_ps[:, J * k : J * (k + 1)],
                        q_sb[:, 128 * k : 128 * (k + 1)],
                        ident[:J, :J],
                    )
                qT_sb = spool.tile([128, NK * J], F32, tag="qt")
                nc.vector.tensor_copy(qT_sb, qt_ps)

                # --- qhatT[d, j] = qT*wm[d] + wc[d] ---
                qhatT = spool.tile([128, NK * J], F32, tag="qhat")
                for k in range(NK):
                    nc.scalar.activation(
                        qhatT[:, J * k : J * (k + 1)],
                        qT_sb[:, J * k : J * (k + 1)],
                        mybir.ActivationFunctionType.Identity,
                        bias=wt_sb[:, k : k + 1],
                        scale=wt_sb[:, 8 + k : 9 + k],
                    )

                # --- q_term[j] = sum_d q[j,d]*wq[d]  (row [1, J]) ---
                qt_acc = ps_vec.tile([1, J], F32, tag="vec")
                for k in range(NK):
                    nc.tensor.matmul(
                        qt_acc,
                        lhsT=wt_sb[:, 4 + k : 5 + k],
                        rhs=qT_sb[:, J * k : J * (k + 1)],
                        start=(k == 0),
                        stop=(k == NK - 1),
                    )
                qtr = small.tile([1, J], F32, tag="qtr")
                nc.scalar.copy(qtr, qt_acc)

                # --- cT blocks: cT[k][:, 128i:128(i+1)] = c[ti, dk].T ---
                cT = []
                for k in range(NK):
                    ct_ps = ps_trans.tile([128, T], F32, tag="trans")
                    for i in range(NT):
                        nc.tensor.transpose(
                            ct_ps[:, 128 * i : 128 * (i + 1)],
                            g[i][:, 128 * k : 128 * (k + 1)],
                            ident,
                        )
                    ct_sb = spool.tile([128, T], F32, tag=f"ct{k}")
                    nc.vector.tensor_copy(ct_sb, ct_ps)
                    cT.append(ct_sb)

                # --- S per t-chunk, then exp / rowmax / rowsum ---
                p_sb = []
                recip = []
                e_sb = small.tile([128, NT], F32, tag="e")
                for i in range(NT):
                    s_ps = ps_s.tile([128, J], F32, tag="s")
                    for k in range(NK):
                        nc.tensor.matmul(
                            s_ps,
                            lhsT=cT[k][:, 128 * i : 128 * (i + 1)],
                            rhs=qhatT[:, J * k : J * (k + 1)],
                            start=(k == 0),
                            stop=False,
                        )
                    nc.tensor.matmul(
                        s_ps, lhsT=ones_row, rhs=qtr, start=False, stop=True
                    )
                    rowmax = small.tile([128, 1], F32, tag="rmax")
                    nc.vector.reduce_max(rowmax, s_ps, axis=mybir.AxisListType.X)
                    nc.scalar.activation(
                        e_sb[:, i : i + 1], rowmax, mybir.ActivationFunctionType.Exp
                    )
                    pt = spool.tile([128, J], F32, tag=f"p{i}")
                    rowsum = small.tile([128, 1], F32, tag="rsum")
                    nc.scalar.activation(
                        pt, s_ps, mybir.ActivationFunctionType.Exp, accum_out=rowsum
                    )
                    rc = small.tile([128, 1], F32, tag=f"rcp{i}")
                    nc.vector.reciprocal(rc, rowsum)
                    p_sb.append(pt)
                    recip.append(rc)

                # --- P^T via PE transpose ---
                pt_ps = ps_trans.tile([J, T], F32, tag="trans")
                for i in range(NT):
                    nc.tensor.transpose(
                        pt_ps[:, 128 * i : 128 * (i + 1)], p_sb[i], ident
                    )
                ptr_sb = spool.tile([J, T], MMDT, tag="pt")
                nc.vector.tensor_copy(ptr_sb, pt_ps)

                # --- c2q per t-chunk; G2 = c2q (normalized), G3 = c * c2q ---
                for i in range(NT):
                    c2q_ps = ps_mm.tile([128, D], F32, tag="mm")
                    nc.tensor.matmul(
                        c2q_ps,
                        lhsT=ptr_sb[:, 128 * i : 128 * (i + 1)],
                        rhs=q_r,
                        start=True,
                        stop=True,
                    )
                    nc.scalar.activation(
                        g[i][:, D : 2 * D],
                        c2q_ps,
                        mybir.ActivationFunctionType.Copy,
                        scale=recip[i],
                    )
                    nc.vector.tensor_mul(
                        g[i][:, 2 * D : 3 * D], g[i][:, D : 2 * D], g[i][:, 0:D]
                    )

                # --- q2c: unnormalized row [1, D] = sum_t e[t] * c[t, :] ---
                q2c_ps = ps_vec.tile([1, D], F32, tag="vec")
                for i in range(NT):
                    nc.tensor.matmul(
                        q2c_ps,
                        lhsT=e_sb[:, i : i + 1],
                        rhs=g[i][:, 0:D],
                        start=(i == 0),
                        stop=(i == NT - 1),
                    )
                # sumexp = sum_t e[t] via per-partition sum then ones matvec
                esum = small.tile([128, 1], F32, tag="esum")
                nc.vector.reduce_sum(esum, e_sb, axis=mybir.AxisListType.X)
                se_ps = ps_vec.tile([1, 1], F32, tag="vec")
                nc.tensor.matmul(se_ps, lhsT=esum, rhs=ones_col, start=True, stop=True)
                rcp_s = small.tile([1, 1], F32, tag="rcps")
                nc.vector.reciprocal(rcp_s, se_ps)
                recip_row = small.tile([1, 128], MMDT, tag="rrow")
                nc.vector.tensor_scalar_mul(recip_row, ones_row, rcp_s)
                q2c_row = small.tile([1, D], MMDT, tag="q2cr")
                nc.scalar.copy(q2c_row, q2c_ps)

                # --- broadcast q2c to all partitions: bc = recip_row^T @ q2c ---
                bc_ps = ps_mm.tile([128, D], F32, tag="mm")
                nc.tensor.matmul(
                    bc_ps,
                    lhsT=recip_row,
                    rhs=q2c_row,
                    start=True,
                    stop=True,
                )

                # --- G4 = c * q2c; store all of G ---
                for i in range(NT):
                    nc.vector.tensor_mul(
                        g[i][:, 3 * D : 4 * D], g[i][:, 0:D], bc_ps
                    )
                    nc.sync.dma_start(
                        out=out_d[b, 128 * i : 128 * (i + 1), :], in_=g[i]
                    )

    # Bacc.compile() splits multi-wait instructions into event-semaphore
    # chains (HW allows at most 1 sync wait per instruction) and runs
    # register allocation / nop fusion before serialization.
    nc.compile()
    return nc


_NC_CACHE = None


def _get_nc():
    global _NC_CACHE
    if _NC_CACHE is None:
        _NC_CACHE = build_kernel()
    return _NC_CACHE


def _prep_in_maps(embd_context, embd_query, W):
    w_resh = np.ascontiguousarray(
        np.asarray(W, dtype=np.float32).reshape(12, 128).T
    )
    in_maps = []
    for c in range(N_CORES):
        sl = slice(c * BPC, (c + 1) * BPC)
        in_maps.append(
            {
                "embd_context": np.ascontiguousarray(
                    np.asarray(embd_context[sl], dtype=np.float32)
                ),
                "embd_query": np.ascontiguousarray(
                    np.asarray(embd_query[sl], dtype=np.float32)
                ),
                "w_resh": w_resh,
            }
        )
    return in_maps


def run_spmd(embd_context, embd_query, W, **spmd_kwargs):
    """Run on all 8 cores; returns (full_output, BassKernelResults)."""
    nc = _get_nc()
    in_maps = _prep_in_maps(embd_context, embd_query, W)
    res = run_bass_kernel_spmd(nc, in_maps, core_ids=list(range(N_CORES)), **spmd_kwargs)
    out = np.concatenate([res.results[c]["g_out"] for c in range(N_CORES)], axis=0)
    return out, res


def kernel(embd_context, embd_query, W):
    out, _ = run_spmd(embd_context, embd_query, W)
    return out


# revision 28
# speedup vs baseline: 1.3624x; 1.3624x over previous
"""Trainium2 Bass kernel for nn_AttentionalFlow (BiDAF-style attention flow).

Reference math (per batch b; c = embd_context [T=512, D=512],
q = embd_query [J=64, D=512], W = [3*D] split into wc, wq, wm):

  S[t,j] = c[t]·wc + q[j]·wq + sum_d c[t,d]*q[j,d]*wm[d]
         = sum_d c[t,d] * (q[j,d]*wm[d] + wc[d]) + q_term[j]
  P      = softmax_j(S)        (softmax is shift-invariant; |S| <~ 8 so we
                                skip the max subtraction and exp directly)
  c2q    = P @ q
  e[t]   = exp(max_j S[t,j]);  q2c[d] = (sum_t e[t]*c[t,d]) / (sum_t e[t])
  G      = [c, c2q, c*c2q, c*q2c]   -> [T, 2048]

Dataflow: compute S^T [j=64, t=512] (contraction over d needs both operands in
[d, .] layout, so c is PE-transposed; the j-on-partitions layout makes every
matmul free dim 512 -> full-rate float32r, lets q_term fold into the exp bias,
and exp(S^T) IS the P^T needed as c2q's stationary operand). e[t] = max_j P
(max of exp = exp of max) and rowsum[t] come from PE-transposing P^T back.
q2c/sumexp/broadcast use matmuls with vector operands (ones / e columns).

Sharding: data-parallel over batch. 32 batches / 8 cores = 4 batches per core.
W is tiny and replicated (pre-reshaped host-side to [128, 12]: col k holds
W[128k:128k+128]; cols 0-3 = wc, 4-7 = wq, 8-11 = wm chunks).
"""

import numpy as np

import concourse.bacc as bacc
import concourse.bass as bass
import concourse.tile as tile
from concourse import mybir
from concourse.bass_utils import run_bass_kernel_spmd
from concourse.masks import make_identity

F32 = mybir.dt.float32
F32R = mybir.dt.float32r
ACTF = mybir.ActivationFunctionType

N_CORES = 8
B, T, J, D = 32, 512, 64, 512
BPC = B // N_CORES  # batches per core
NT = T // 128       # t-chunks of 128
NK = D // 128       # d-chunks of 128
GD = 4 * D          # output feature dim

# float32r: fp32 matmuls at full rate (vs 1/4) when the moving free dim >=256.
# Operands must be materialized as rounded fp32r by their producers.
USE_F32R = True
MMDT = F32R if USE_F32R else F32


def build_kernel():
    nc = bacc.Bacc()

    ctx_d = nc.dram_tensor("embd_context", [BPC, T, D], F32, kind="ExternalInput")
    qry_d = nc.dram_tensor("embd_query", [BPC, J, D], F32, kind="ExternalInput")
    wt_d = nc.dram_tensor("w_resh", [128, 12], F32, kind="ExternalInput")
    out_d = nc.dram_tensor("g_out", [BPC, T, GD], F32, kind="ExternalOutput")

    with tile.TileContext(nc) as tc:
        with (
            tc.tile_pool(name="singles", bufs=1) as singles,
            tc.tile_pool(name="gpool", bufs=3) as gpool,
            tc.tile_pool(name="spool", bufs=2) as spool,
            tc.tile_pool(name="small", bufs=8) as small,
            tc.tile_pool(name="ps_trans", bufs=3, space="PSUM") as ps_trans,
            tc.tile_pool(name="ps_s", bufs=1, space="PSUM") as ps_s,
            tc.tile_pool(name="ps_mm", bufs=2, space="PSUM") as ps_mm,
            tc.tile_pool(name="ps_vec", bufs=2, space="PSUM") as ps_vec,
        ):
            ident = singles.tile([128, 128], F32)
            make_identity(nc, ident)
            ones_row = singles.tile([1, 128], F32)
            nc.vector.memset(ones_row, 1.0)
            ones_col = singles.tile([128, 1], F32)
            nc.vector.memset(ones_col, 1.0)
            # fp32r constants must come from compute ops (rounded producers)
            ident_r = singles.tile([128, 128], MMDT)
            nc.vector.tensor_copy(ident_r, ident)
            ones_row_r = singles.tile([1, 128], MMDT)
            nc.vector.tensor_copy(ones_row_r, ones_row)
            ones_col_r = singles.tile([128, 1], MMDT)
            nc.vector.tensor_copy(ones_col_r, ones_col)
            wt_sb = singles.tile([128, 12], F32)
            nc.gpsimd.dma_start(out=wt_sb, in_=wt_d[:, :])

            for b in range(BPC):
                # --- load context into the G output tiles (slot 0 = c) ---
                g = [
                    gpool.tile([128, GD], F32, tag=f"g{i}", name=f"g{i}")
                    for i in range(NT)
                ]
                c_r = []
                for i in range(NT):
                    nc.gpsimd.dma_start(
                        out=g[i][:, 0:D],
                        in_=ctx_d[b, 128 * i : 128 * (i + 1), :],
                    )
                    # G slot 0 is just a copy of c: stream it back out
                    # immediately so 25% of the output bytes overlap compute.
                    nc.sync.dma_start(
                        out=out_d[b, 128 * i : 128 * (i + 1), 0:D],
                        in_=g[i][:, 0:D],
                    )
                    if USE_F32R:
                        cri = spool.tile(
                            [128, D], MMDT, tag=f"cr{i}", name=f"cr{i}"
                        )
                        nc.vector.tensor_copy(cri, g[i][:, 0:D])
                        c_r.append(cri)
                    else:
                        c_r.append(g[i][:, 0:D])
                q_sb = spool.tile([J, D], F32, tag="q")
                nc.gpsimd.dma_start(out=q_sb, in_=qry_d[b])
                if USE_F32R:
                    q_r = spool.tile([J, D], MMDT, tag="qr")
                    nc.vector.tensor_copy(q_r, q_sb)
                else:
                    q_r = q_sb

                # --- qT: [d, j] blocks via PE transpose ---
                qt_ps = ps_trans.tile([128, NK * J], F32, tag="trans")
                for k in range(NK):
                    nc.tensor.transpose(
                        qt_ps[:, J * k : J * (k + 1)],
                        q_sb[:, 128 * k : 128 * (k + 1)],
                        ident[:J, :J],
                    )
                qT_sb = spool.tile([128, NK * J], F32, tag="qt")
                nc.any.tensor_copy(qT_sb, qt_ps)

                # --- qhatT[d, j] = qT*wm[d] + wc[d] (rounded for matmul) ---
                qhatT = spool.tile([128, NK * J], MMDT, tag="qhat")
                for k in range(NK):
                    nc.scalar.activation(
                        qhatT[:, J * k : J * (k + 1)],
                        qT_sb[:, J * k : J * (k + 1)],
                        ACTF.Identity,
                        bias=wt_sb[:, k : k + 1],
                        scale=wt_sb[:, 8 + k : 9 + k],
                    )

                # --- q_term column [J, 1]: folded into the exp bias below ---
                qt_ps2 = ps_vec.tile([J, 1], F32, tag="vec")
                for k in range(NK):
                    nc.tensor.matmul(
                        qt_ps2,
                        lhsT=qT_sb[:, J * k : J * (k + 1)],
                        rhs=wt_sb[:, 4 + k : 5 + k],
                        start=(k == 0),
                        stop=(k == NK - 1),
                    )
                qt_col = small.tile([J, 1], F32, tag="qtc")
                nc.scalar.copy(qt_col, qt_ps2)

                # --- cT blocks: cT[k][:, 128i:128(i+1)] = c[ti, dk].T ---
                cT = []
                for k in range(NK):
                    ct_ps = ps_trans.tile([128, T], F32, tag="trans")
                    for i in range(NT):
                        nc.tensor.transpose(
                            ct_ps[:, 128 * i : 128 * (i + 1)],
                            g[i][:, 128 * k : 128 * (k + 1)],
                            ident,
                        )
                    ct_sb = spool.tile([128, T], MMDT, tag=f"ct{k}", name=f"ct{k}")
                    nc.any.tensor_copy(ct_sb, ct_ps)
                    cT.append(ct_sb)

                # --- S^T [j, t] = qhatT.T @ cT  (full-rate f32r, N=512) ---
                st_ps = ps_s.tile([J, T], F32, tag="s")
                for k in range(NK):
                    nc.tensor.matmul(
                        st_ps,
                        lhsT=qhatT[:, J * k : J * (k + 1)],
                        rhs=cT[k],
                        start=(k == 0),
                        stop=(k == NK - 1),
                    )
                # P^T = exp(S^T + q_term[j]); this is c2q's stationary operand
                ptr_sb = spool.tile([J, T], MMDT, tag="pt")
                nc.scalar.activation(
                    ptr_sb, st_ps, ACTF.Exp, bias=qt_col, scale=1.0
                )

                # --- c2q per t-chunk; G2 = c2q/rowsum, G3 = c * c2q ---
                # P back in [t, j] layout (one PSUM bank) for rowmax/rowsum
                pall_ps = ps_trans.tile([128, NT * J], MMDT, tag="trans")
                for i in range(NT):
                    nc.tensor.transpose(
                        pall_ps[:, J * i : J * (i + 1)],
                        ptr_sb[:, 128 * i : 128 * (i + 1)],
                        ident_r[:J, :J],
                    )
                # e[t] = max_j P (exp of max == max of exp); rowsum for softmax
                e_sb = small.tile([128, NT], MMDT, tag="e")
                nc.vector.reduce_max(
                    e_sb,
                    pall_ps.rearrange("p (n j) -> p n j", j=J),
                    axis=mybir.AxisListType.X,
                )
                rs_sb = small.tile([128, NT], F32, tag="rs")
                nc.vector.reduce_sum(
                    rs_sb,
                    pall_ps.rearrange("p (n j) -> p n j", j=J),
                    axis=mybir.AxisListType.X,
                )
                recip = small.tile([128, NT], F32, tag="rcp")
                nc.vector.reciprocal(recip, rs_sb)

                for i in range(NT):
                    c2q_ps = ps_mm.tile([128, D], F32, tag="mm")
                    nc.tensor.matmul(
                        c2q_ps,
                        lhsT=ptr_sb[:, 128 * i : 128 * (i + 1)],
                        rhs=q_r,
                        start=True,
                        stop=True,
                    )
                    nc.scalar.activation(
                        g[i][:, D : 2 * D],
                        c2q_ps,
                        ACTF.Copy,
                        scale=recip[:, i : i + 1],
                    )
                    # all-SBUF multiply: run on the otherwise-idle GPSIMD
                    nc.gpsimd.tensor_mul(
                        g[i][:, 2 * D : 3 * D], g[i][:, D : 2 * D], g[i][:, 0:D]
                    )
                    # stream out the middle strip as soon as G2/G3 are ready
                    nc.sync.dma_start(
                        out=out_d[b, 128 * i : 128 * (i + 1), D : 3 * D],
                        in_=g[i][:, D : 3 * D],
                    )

                # --- q2c row [1, D] = sum_t e[t]*c[t,:]; sumexp the same way ---
                q2c_ps = ps_vec.tile([1, D], F32, tag="vec")
                for i in range(NT):
                    nc.tensor.matmul(
                        q2c_ps,
                        lhsT=e_sb[:, i : i + 1],
                        rhs=c_r[i],
                        start=(i == 0),
                        stop=(i == NT - 1),
                    )
                # sumexp: per-partition sum of e then a single f32 matvec
                # (fp32r is not ISA-legal at free dim 1)
                esum = small.tile([128, 1], F32, tag="esum")
                nc.vector.reduce_sum(esum, e_sb, axis=mybir.AxisListType.X)
                se_ps = ps_vec.tile([1, 1], F32, tag="vec")
                nc.tensor.matmul(
                    se_ps, lhsT=esum, rhs=ones_col, start=True, stop=True
                )
                rcp_s = small.tile([1, 1], F32, tag="rcps")
                nc.vector.reciprocal(rcp_s, se_ps)
                # normalized q2c row in one fused op (scalar ptr broadcast)
                q2c_row = small.tile([1, D], MMDT, tag="q2cr")
                nc.vector.tensor_scalar_mul(q2c_row, q2c_ps, rcp_s)

                # --- broadcast q2c to all partitions: bc = ones^T @ q2c ---
                bc_ps = ps_mm.tile([128, D], F32, tag="mm")
                nc.tensor.matmul(
                    bc_ps, lhsT=ones_row_r, rhs=q2c_row, start=True, stop=True
                )

                # --- G4 = c * q2c; store the final strip ---
                for i in range(NT):
                    nc.vector.tensor_mul(
                        g[i][:, 3 * D : 4 * D], g[i][:, 0:D], bc_ps
                    )
                    nc.sync.dma_start(
                        out=out_d[b, 128 * i : 128 * (i + 1), 3 * D : 4 * D],
                        in_=g[i][:, 3 * D : 4 * D],
                    )

    # Bacc.compile() splits multi-wait instructions into event-semaphore
    # chains (HW allows at most 1 sync wait per instruction) and runs
    # register allocation / nop fusion before serialization.
    nc.compile()
    return nc


_NC_CACHE = None


def _get_nc():
    global _NC_CACHE
    if _NC_CACHE is None:
        _NC_CACHE = build_kernel()
    return _NC_CACHE


def _prep_in_maps(embd_context, embd_query, W):
    w_resh = np.ascontiguousarray(
        np.asarray(W, dtype=np.float32).reshape(12, 128).T
    )
    in_maps = []
    for c in range(N_CORES):
        sl = slice(c * BPC, (c + 1) * BPC)
        in_maps.append(
            {
                "embd_context": np.ascontiguousarray(
                    np.asarray(embd_context[sl], dtype=np.float32)
                ),
                "embd_query": np.ascontiguousarray(
                    np.asarray(embd_query[sl], dtype=np.float32)
                ),
                "w_resh": w_resh,
            }
        )
    return in_maps


def run_spmd(embd_context, embd_query, W, **spmd_kwargs):
    """Run on all 8 cores; returns (full_output, BassKernelResults)."""
    nc = _get_nc()
    in_maps = _prep_in_maps(embd_context, embd_query, W)
    res = run_bass_kernel_spmd(nc, in_maps, core_ids=list(range(N_CORES)), **spmd_kwargs)
    out = np.concatenate([res.results[c]["g_out"] for c in range(N_CORES)], axis=0)
    return out, res


def kernel(embd_context, embd_query, W):
    out, _ = run_spmd(embd_context, embd_query, W)
    return out


# revision 34
# speedup vs baseline: 93240.0301x; 68438.9293x over previous
"""Trainium2 Bass kernel for nn_AttentionalFlow (BiDAF-style attention flow).

Reference math (per batch b; c = embd_context [T=512, D=512],
q = embd_query [J=64, D=512], W = [3*D] split into wc, wq, wm):

  S[t,j] = c[t]·wc + q[j]·wq + sum_d c[t,d]*q[j,d]*wm[d]
         = sum_d c[t,d] * (q[j,d]*wm[d] + wc[d]) + q_term[j]
  P      = softmax_j(S)        (softmax is shift-invariant; |S| <~ 8 so we
                                skip the max subtraction and exp directly)
  c2q    = P @ q
  e[t]   = exp(max_j S[t,j]);  q2c[d] = (sum_t e[t]*c[t,d]) / (sum_t e[t])
  G      = [c, c2q, c*c2q, c*q2c]   -> [T, 2048]

Dataflow: compute S^T [j=64, t=512] (contraction over d needs both operands in
[d, .] layout, so c is PE-transposed; the j-on-partitions layout makes every
matmul free dim 512 -> full-rate float32r, lets q_term fold into the exp bias,
and exp(S^T) IS the P^T needed as c2q's stationary operand). e[t] = max_j P
(max of exp = exp of max) and rowsum[t] come from PE-transposing P^T back.
q2c/sumexp/broadcast use matmuls with vector operands (ones / e columns).

Sharding: data-parallel over batch. 32 batches / 8 cores = 4 batches per core.
W is tiny and replicated (pre-reshaped host-side to [128, 12]: col k holds
W[128k:128k+128]; cols 0-3 = wc, 4-7 = wq, 8-11 = wm chunks).
"""

import contextlib

import numpy as np

import concourse.bacc as bacc
import concourse.bass as bass
import concourse.tile as tile
from concourse import mybir
from concourse.bass_utils import run_bass_kernel_spmd
from concourse.masks import make_identity

F32 = mybir.dt.float32
F32R = mybir.dt.float32r
ACTF = mybir.ActivationFunctionType

N_CORES = 8
B, T, J, D = 32, 512, 64, 512
BPC = B // N_CORES  # batches per core
NT = T // 128       # t-chunks of 128
NK = D // 128       # d-chunks of 128
GD = 4 * D          # output feature dim

# float32r: fp32 matmuls at full rate (vs 1/4) when the moving free dim >=256.
# Operands must be materialized as rounded fp32r by their producers.
USE_F32R = True
MMDT = F32R if USE_F32R else F32


def build_kernel(loop_reps=None):
    """loop_reps: if set, wrap the whole body in a HW For_i loop that
    re-executes it that many times (used only for timing measurement —
    amplifies device time so axon dispatch jitter can be differenced out)."""
    nc = bacc.Bacc()

    ctx_d = nc.dram_tensor("embd_context", [BPC, T, D], F32, kind="ExternalInput")
    qry_d = nc.dram_tensor("embd_query", [BPC, J, D], F32, kind="ExternalInput")
    wt_d = nc.dram_tensor("w_resh", [128, 12], F32, kind="ExternalInput")
    out_d = nc.dram_tensor("g_out", [BPC, T, GD], F32, kind="ExternalOutput")

    with tile.TileContext(nc) as tc:
        with (
            tc.tile_pool(name="singles", bufs=1) as singles,
            tc.tile_pool(name="gpool", bufs=3) as gpool,
            tc.tile_pool(name="spool", bufs=2) as spool,
            tc.tile_pool(name="small", bufs=8) as small,
            tc.tile_pool(name="ps_trans", bufs=3, space="PSUM") as ps_trans,
            tc.tile_pool(name="ps_s", bufs=1, space="PSUM") as ps_s,
            tc.tile_pool(name="ps_mm", bufs=2, space="PSUM") as ps_mm,
            tc.tile_pool(name="ps_vec", bufs=2, space="PSUM") as ps_vec,
        ):
            ident = singles.tile([128, 128], F32)
            make_identity(nc, ident)
            ones_row = singles.tile([1, 128], F32)
            nc.vector.memset(ones_row, 1.0)
            ones_col = singles.tile([128, 1], F32)
            nc.vector.memset(ones_col, 1.0)
            # fp32r constants must come from compute ops (rounded producers)
            ident_r = singles.tile([128, 128], MMDT)
            nc.vector.tensor_copy(ident_r, ident)
            ones_row_r = singles.tile([1, 128], MMDT)
            nc.vector.tensor_copy(ones_row_r, ones_row)
            ones_col_r = singles.tile([128, 1], MMDT)
            nc.vector.tensor_copy(ones_col_r, ones_col)
            wt_sb = singles.tile([128, 12], F32)
            nc.gpsimd.dma_start(out=wt_sb, in_=wt_d[:, :])

            loop_cm = (
                tc.For_i(0, loop_reps, 1)
                if loop_reps is not None
                else contextlib.nullcontext()
            )
            with loop_cm:
              for b in range(BPC):
                # --- load context into the G output tiles (slot 0 = c) ---
                g = [
                    gpool.tile([128, GD], F32, tag=f"g{i}", name=f"g{i}")
                    for i in range(NT)
                ]
                c_r = []
                for i in range(NT):
                    nc.gpsimd.dma_start(
                        out=g[i][:, 0:D],
                        in_=ctx_d[b, 128 * i : 128 * (i + 1), :],
                    )
                    # G slot 0 is just a copy of c: stream it back out
                    # immediately so 25% of the output bytes overlap compute.
                    nc.sync.dma_start(
                        out=out_d[b, 128 * i : 128 * (i + 1), 0:D],
                        in_=g[i][:, 0:D],
                    )
                    if USE_F32R:
                        cri = spool.tile(
                            [128, D], MMDT, tag=f"cr{i}", name=f"cr{i}"
                        )
                        nc.vector.tensor_copy(cri, g[i][:, 0:D])
                        c_r.append(cri)
                    else:
                        c_r.append(g[i][:, 0:D])
                q_sb = spool.tile([J, D], F32, tag="q")
                nc.gpsimd.dma_start(out=q_sb, in_=qry_d[b])
                if USE_F32R:
                    q_r = spool.tile([J, D], MMDT, tag="qr")
                    nc.vector.tensor_copy(q_r, q_sb)
                else:
                    q_r = q_sb

                # --- qT: [d, j] blocks via PE transpose ---
                qt_ps = ps_trans.tile([128, NK * J], F32, tag="trans")
                for k in range(NK):
                    nc.tensor.transpose(
                        qt_ps[:, J * k : J * (k + 1)],
                        q_sb[:, 128 * k : 128 * (k + 1)],
                        ident[:J, :J],
                    )
                qT_sb = spool.tile([128, NK * J], F32, tag="qt")
                nc.any.tensor_copy(qT_sb, qt_ps)

                # --- qhatT[d, j] = qT*wm[d] + wc[d] (rounded for matmul) ---
                qhatT = spool.tile([128, NK * J], MMDT, tag="qhat")
                for k in range(NK):
                    nc.scalar.activation(
                        qhatT[:, J * k : J * (k + 1)],
                        qT_sb[:, J * k : J * (k + 1)],
                        ACTF.Identity,
                        bias=wt_sb[:, k : k + 1],
                        scale=wt_sb[:, 8 + k : 9 + k],
                    )

                # --- q_term column [J, 1]: folded into the exp bias below ---
                qt_ps2 = ps_vec.tile([J, 1], F32, tag="vec")
                for k in range(NK):
                    nc.tensor.matmul(
                        qt_ps2,
                        lhsT=qT_sb[:, J * k : J * (k + 1)],
                        rhs=wt_sb[:, 4 + k : 5 + k],
                        start=(k == 0),
                        stop=(k == NK - 1),
                    )
                qt_col = small.tile([J, 1], F32, tag="qtc")
                nc.scalar.copy(qt_col, qt_ps2)

                # --- cT blocks: cT[k][:, 128i:128(i+1)] = c[ti, dk].T ---
                cT = []
                for k in range(NK):
                    ct_ps = ps_trans.tile([128, T], F32, tag="trans")
                    for i in range(NT):
                        nc.tensor.transpose(
                            ct_ps[:, 128 * i : 128 * (i + 1)],
                            g[i][:, 128 * k : 128 * (k + 1)],
                            ident,
                        )
                    ct_sb = spool.tile([128, T], MMDT, tag=f"ct{k}", name=f"ct{k}")
                    nc.any.tensor_copy(ct_sb, ct_ps)
                    cT.append(ct_sb)

                # --- S^T [j, t] = qhatT.T @ cT  (full-rate f32r, N=512) ---
                st_ps = ps_s.tile([J, T], F32, tag="s")
                for k in range(NK):
                    nc.tensor.matmul(
                        st_ps,
                        lhsT=qhatT[:, J * k : J * (k + 1)],
                        rhs=cT[k],
                        start=(k == 0),
                        stop=(k == NK - 1),
                    )
                # P^T = exp(S^T + q_term[j]); this is c2q's stationary operand
                ptr_sb = spool.tile([J, T], MMDT, tag="pt")
                nc.scalar.activation(
                    ptr_sb, st_ps, ACTF.Exp, bias=qt_col, scale=1.0
                )

                # --- P back in [t, j] layout for rowmax/rowsum ---
                pall_ps = ps_trans.tile([128, NT * J], MMDT, tag="trans")
                for i in range(NT):
                    nc.tensor.transpose(
                        pall_ps[:, J * i : J * (i + 1)],
                        ptr_sb[:, 128 * i : 128 * (i + 1)],
                        ident_r[:J, :J],
                    )
                # e[t] = max_j P (exp of max == max of exp); rowsum for softmax
                e_sb = small.tile([128, NT], MMDT, tag="e")
                nc.vector.reduce_max(
                    e_sb,
                    pall_ps.rearrange("p (n j) -> p n j", j=J),
                    axis=mybir.AxisListType.X,
                )
                rs_sb = small.tile([128, NT], F32, tag="rs")
                nc.vector.reduce_sum(
                    rs_sb,
                    pall_ps.rearrange("p (n j) -> p n j", j=J),
                    axis=mybir.AxisListType.X,
                )
                recip = small.tile([128, NT], F32, tag="rcp")
                nc.vector.reciprocal(recip, rs_sb)

                # --- c2q per t-chunk; G2 = c2q/rowsum, G3 = c * c2q ---
                for i in range(NT):
                    c2q_ps = ps_mm.tile([128, D], F32, tag="mm")
                    nc.tensor.matmul(
                        c2q_ps,
                        lhsT=ptr_sb[:, 128 * i : 128 * (i + 1)],
                        rhs=q_r,
                        start=True,
                        stop=True,
                    )
                    nc.scalar.activation(
                        g[i][:, D : 2 * D],
                        c2q_ps,
                        ACTF.Copy,
                        scale=recip[:, i : i + 1],
                    )
                    # all-SBUF multiply: run on the otherwise-idle GPSIMD
                    nc.gpsimd.tensor_mul(
                        g[i][:, 2 * D : 3 * D], g[i][:, D : 2 * D], g[i][:, 0:D]
                    )
                    # stream out the middle strip as soon as G2/G3 are ready
                    nc.sync.dma_start(
                        out=out_d[b, 128 * i : 128 * (i + 1), D : 3 * D],
                        in_=g[i][:, D : 3 * D],
                    )

                # --- q2c row [1, D] = sum_t e[t]*c[t,:] ---
                q2c_ps = ps_vec.tile([1, D], F32, tag="vec")
                for i in range(NT):
                    nc.tensor.matmul(
                        q2c_ps,
                        lhsT=e_sb[:, i : i + 1],
                        rhs=c_r[i],
                        start=(i == 0),
                        stop=(i == NT - 1),
                    )
                # sumexp: per-partition sum of e then a single f32 matvec
                # (fp32r is not ISA-legal at free dim 1)
                esum = small.tile([128, 1], F32, tag="esum")
                nc.vector.reduce_sum(esum, e_sb, axis=mybir.AxisListType.X)
                se_ps = ps_vec.tile([1, 1], F32, tag="vec")
                nc.tensor.matmul(
                    se_ps, lhsT=esum, rhs=ones_col, start=True, stop=True
                )
                rcp_s = small.tile([1, 1], F32, tag="rcps")
                nc.vector.reciprocal(rcp_s, se_ps)
                # normalized q2c row in one fused op (scalar ptr broadcast)
                q2c_row = small.tile([1, D], MMDT, tag="q2cr")
                nc.vector.tensor_scalar_mul(q2c_row, q2c_ps, rcp_s)

                # --- broadcast q2c to all partitions: bc = ones^T @ q2c ---
                bc_ps = ps_mm.tile([128, D], F32, tag="mm")
                nc.tensor.matmul(
                    bc_ps, lhsT=ones_row_r, rhs=q2c_row, start=True, stop=True
                )

                # --- G4 = c * q2c; store the final strip ---
                for i in range(NT):
                    nc.vector.tensor_mul(
                        g[i][:, 3 * D : 4 * D], g[i][:, 0:D], bc_ps
                    )
                    nc.sync.dma_start(
                        out=out_d[b, 128 * i : 128 * (i + 1), 3 * D : 4 * D],
                        in_=g[i][:, 3 * D : 4 * D],
                    )

    # Bacc.compile() splits multi-wait instructions into event-semaphore
    # chains (HW allows at most 1 sync wait per instruction) and runs
    # register allocation / nop fusion before serialization.
    nc.compile()
    return nc


_NC_CACHE = None


def _get_nc():
    global _NC_CACHE
    if _NC_CACHE is None:
        _NC_CACHE = build_kernel()
    return _NC_CACHE


def _prep_in_maps(embd_context, embd_query, W):
    w_resh = np.ascontiguousarray(
        np.asarray(W, dtype=np.float32).reshape(12, 128).T
    )
    in_maps = []
    for c in range(N_CORES):
        sl = slice(c * BPC, (c + 1) * BPC)
        in_maps.append(
            {
                "embd_context": np.ascontiguousarray(
                    np.asarray(embd_context[sl], dtype=np.float32)
                ),
                "embd_query": np.ascontiguousarray(
                    np.asarray(embd_query[sl], dtype=np.float32)
                ),
                "w_resh": w_resh,
            }
        )
    return in_maps


def run_spmd(embd_context, embd_query, W, **spmd_kwargs):
    """Run on all 8 cores; returns (full_output, BassKernelResults)."""
    nc = _get_nc()
    in_maps = _prep_in_maps(embd_context, embd_query, W)
    res = run_bass_kernel_spmd(nc, in_maps, core_ids=list(range(N_CORES)), **spmd_kwargs)
    out = np.concatenate([res.results[c]["g_out"] for c in range(N_CORES)], axis=0)
    return out, res


def kernel(embd_context, embd_query, W):
    out, _ = run_spmd(embd_context, embd_query, W)
    return out
